# revision 1
# baseline (speedup 1.0000x reference)
"""Trainium2 Bass kernel for AttentionPatcher (GQA attention block, S=2048).

Sharding: 8-way tensor parallel over KV head groups. Core c owns KV head c
and query heads 4c..4c+3: it computes its Q/K/V projections, RoPE, causal
attention, and a full partial o_proj (wo column shard); a ReduceScatter(add)
over the 8 cores then leaves core c with rows [512c, 512c+512) of the final
output, which the host concatenates.

All matmuls run as float32r (full-rate fp32-reduced) on the PE.
"""
import os
import sys

import numpy as np

if os.path.isdir("/opt/trn_rl_repo") and "/opt/trn_rl_repo" not in sys.path:
    sys.path.insert(0, "/opt/trn_rl_repo")

import concourse.bacc as bacc
import concourse.mybir as mybir
import concourse.tile as tile
from concourse.bass_utils import run_bass_kernel_spmd
from concourse.masks import make_identity

F32 = mybir.dt.float32
F32R = mybir.dt.float32r
ActF = mybir.ActivationFunctionType
Alu = mybir.AluOpType

H, KV, D, S = 32, 8, 128, 2048
HID = H * D
NCORES = 8
G = H // KV          # query heads per core
ST = 512             # s-tile size
NST = S // ST        # 4 s-tiles
KO = HID // 128      # 32 contraction subtiles
MO = HID // 128      # 32 output row tiles
INV_SQRT_D = 1.0 / float(np.sqrt(D))


def build_nc(with_collective=True):
    nc = bacc.Bacc("TRN2", target_bir_lowering=False, debug=False)

    x = nc.dram_tensor("x", [KO, 128, S], F32R, kind="ExternalInput")
    wq = nc.dram_tensor("wq", [KO, 128, G * 128], F32R, kind="ExternalInput")
    wk = nc.dram_tensor("wk", [128, KO, 128], F32R, kind="ExternalInput")
    wv = nc.dram_tensor("wv", [128, KO, 128], F32R, kind="ExternalInput")
    wo = nc.dram_tensor("wo", [MO, 128, G, 128], F32R, kind="ExternalInput")
    bq = nc.dram_tensor("bq", [128, G], F32, kind="ExternalInput")
    bk = nc.dram_tensor("bk", [128, 1], F32, kind="ExternalInput")
    bv = nc.dram_tensor("bv", [128, 1], F32, kind="ExternalInput")
    cos = nc.dram_tensor("cos", [128, S], F32, kind="ExternalInput")
    sin = nc.dram_tensor("sin", [128, S], F32, kind="ExternalInput")
    rot = nc.dram_tensor("rot", [128, 128], F32R, kind="ExternalInput")
    yout = nc.dram_tensor("y", [G, 128, S], F32, kind="ExternalOutput")

    with tile.TileContext(nc) as tc:
        with (
            tc.tile_pool(name="const", bufs=1) as const,
            tc.tile_pool(name="sb", bufs=3) as sb,
            tc.tile_pool(name="ps", bufs=1, space="PSUM") as ps,
            tc.tile_pool(name="dram", bufs=1, space="DRAM") as dram,
        ):
            # ---- resident constants ----
            # constants go through the scalar/gpsimd DMA queues (chunked)
            # so the sync queue starts streaming x/wq immediately
            wk_sb = const.tile([128, KO, 128], F32R)
            wv_sb = const.tile([128, KO, 128], F32R)
            for c8 in range(8):
                ksl = slice(c8 * (KO // 8), (c8 + 1) * (KO // 8))
                nc.scalar.dma_start(wk_sb[:, ksl, :], wk[:, ksl, :])
                nc.scalar.dma_start(wv_sb[:, ksl, :], wv[:, ksl, :])
            cos_sb = const.tile([128, S], F32)
            sin_sb = const.tile([128, S], F32)
            nc.scalar.dma_start(cos_sb[:], cos[:, :])
            nc.gpsimd.dma_start(sin_sb[:], sin[:, :])
            rot_sb = const.tile([128, 128], F32R)
            nc.scalar.dma_start(rot_sb[:], rot[:, :])
            bq_sb = const.tile([128, G], F32)
            bk_sb = const.tile([128, 1], F32)
            bv_sb = const.tile([128, 1], F32)
            nc.scalar.dma_start(bq_sb[:], bq[:, :])
            nc.scalar.dma_start(bk_sb[:], bk[:, :])
            nc.scalar.dma_start(bv_sb[:], bv[:, :])
            ones_f = const.tile([128, 128], F32)
            nc.any.memset(ones_f[:], 1.0)
            ones_r = const.tile([128, 128], F32R)
            nc.vector.tensor_copy(ones_r[:], ones_f[:])
            ident = const.tile([128, 128], F32)
            make_identity(nc, ident)

            # ---- resident activations ----
            k_rot = const.tile([128, S], F32R)          # K, (d, l) layout
            v_t = const.tile([128, S // 128, 128], F32R)  # V^T, (l % 128, l//128, d)
            # attn out, one tile per (g, si) so o_proj deps stay per-slice
            out_t = [[const.tile([128, ST], F32R, name=f"out_{g}_{si}")
                      for si in range(NST)] for g in range(G)]

            def rope(raw_r, dst_ap, sl):
                """dst = raw*cos + (rot@raw)*sin over s-slice sl."""
                ps_rot = ps.tile([128, ST], F32, tag="mm", bufs=4, name="ps_rot")
                nc.tensor.matmul(ps_rot[:], rot_sb[:], raw_r[:],
                                 start=True, stop=True)
                t1 = sb.tile([128, ST], F32, tag="rope_t1", bufs=2)
                t2 = sb.tile([128, ST], F32, tag="rope_t2", bufs=2)
                nc.vector.tensor_tensor(t1[:], raw_r[:], cos_sb[:, sl], Alu.mult)
                nc.vector.tensor_tensor(t2[:], ps_rot[:], sin_sb[:, sl], Alu.mult)
                nc.vector.tensor_tensor(dst_ap, t1[:], t2[:], Alu.add)

            for si in range(NST):
                sl = slice(si * ST, (si + 1) * ST)
                # ---------- QKV projections for this s-tile ----------
                ps_q = [ps.tile([128, ST], F32, tag="mm", bufs=4, name=f"ps_q{g}")
                        for g in range(G)]
                ps_kv = ps.tile([128, 2 * ST], F32, tag="big", bufs=2)
                ps_k = ps_kv[:, 0:ST]
                ps_v = ps_kv[:, ST:2 * ST]
                for ko in range(KO):
                    xt = sb.tile([128, ST], F32R, tag="x", bufs=8)
                    nc.sync.dma_start(xt[:], x[ko][:, sl])
                    wqt = sb.tile([128, G * 128], F32R, tag="wq", bufs=12)
                    nc.sync.dma_start(wqt[:], wq[ko])
                    st = (ko == 0)
                    sp = (ko == KO - 1)
                    for g in range(G):
                        nc.tensor.matmul(ps_q[g][:],
                                         wqt[:, g * 128:(g + 1) * 128], xt[:],
                                         start=st, stop=sp)
                    nc.tensor.matmul(ps_k, wk_sb[:, ko, :], xt[:],
                                     start=st, stop=sp)
                    nc.tensor.matmul(ps_v, wv_sb[:, ko, :], xt[:],
                                     start=st, stop=sp)

                # ---------- K: bias + rope into resident k_rot ----------
                k_raw = sb.tile([128, ST], F32R, tag="k_raw", bufs=2)
                nc.vector.tensor_scalar(k_raw[:], ps_k, bk_sb[:, 0:1], None,
                                        Alu.add)
                rope(k_raw, k_rot[:, sl], sl)

                # ---------- V: bias, then transpose into v_t ----------
                v_sb = sb.tile([128, ST], F32, tag="v_sb", bufs=2)
                nc.vector.tensor_scalar(v_sb[:], ps_v, bv_sb[:, 0:1], None,
                                        Alu.add)
                for j in range(ST // 128):
                    ps_t = ps.tile([128, 128], F32, tag="mm", bufs=4)
                    nc.tensor.transpose(ps_t[:], v_sb[:, j * 128:(j + 1) * 128],
                                        ident[:])
                    nc.vector.tensor_copy(v_t[:, si * (ST // 128) + j, :], ps_t[:])

                # ---------- rope all 4 query heads up front ----------
                nli = (si + 1) * (ST // 128)  # visible l-blocks
                q_rots = []
                for g in range(G):
                    q_raw = sb.tile([128, ST], F32R, tag="q_raw", bufs=2,
                                    name=f"q_raw{g}")
                    nc.vector.tensor_scalar(q_raw[:], ps_q[g][:],
                                            bq_sb[:, g:g + 1], INV_SQRT_D,
                                            Alu.add, Alu.mult)
                    q_rot = sb.tile([128, ST], F32R, tag="q_rot", bufs=4,
                                    name=f"q_rot{g}")
                    rope(q_raw, q_rot[:], sl)
                    q_rots.append(q_rot)

                # ---------- attention per query head ----------
                for g in range(G):
                    q_rot = q_rots[g]
                    ps_av = ps.tile([128, ST], F32, tag="mm", bufs=4)
                    ps_den = ps.tile([128, ST], F32, tag="mm", bufs=4)
                    for pi in range(nli // 2):
                        ps_s2 = ps.tile([128, 2 * ST], F32, tag="big", bufs=2)
                        offs = []
                        for h in range(2):
                            li = 2 * pi + h
                            j = li - si * (ST // 128)
                            # diagonal block j: columns [0, 128j) are fully
                            # masked -> skip them (affine_select zeroes the
                            # garbage left in psum/p there)
                            off = 128 * j if j > 0 else 0
                            offs.append(off)
                            nc.tensor.matmul(
                                ps_s2[:, h * ST + off:(h + 1) * ST],
                                k_rot[:, li * 128:(li + 1) * 128],
                                q_rot[:, off:], start=True, stop=True)
                        p2 = sb.tile([128, 2 * ST], F32R, tag="p", bufs=3)
                        nc.scalar.activation(p2[:], ps_s2[:], ActF.Exp)
                        for h in range(2):
                            li = 2 * pi + h
                            j = li - si * (ST // 128)
                            if j >= 0:
                                # causal: keep where l <= s (ds - dl - 128j >= 0)
                                nc.gpsimd.affine_select(
                                    out=p2[:, h * ST:(h + 1) * ST],
                                    in_=p2[:, h * ST:(h + 1) * ST],
                                    compare_op=Alu.is_ge, fill=0.0,
                                    base=-128 * j, channel_multiplier=-1,
                                    pattern=[[1, ST]],
                                )
                        for h in range(2):
                            li = 2 * pi + h
                            off = offs[h]
                            ph = p2[:, h * ST + off:(h + 1) * ST]
                            nc.tensor.matmul(ps_av[:, off:], v_t[:, li, :], ph,
                                             start=(li == 0),
                                             stop=(li == nli - 1))
                            nc.tensor.matmul(ps_den[:, off:], ones_r[:], ph,
                                             start=(li == 0),
                                             stop=(li == nli - 1))
                    # evict accumulators fast so their PSUM slots recycle;
                    # the slow reciprocal then runs off the critical path
                    den_sb = sb.tile([128, ST], F32, tag="den_sb", bufs=2)
                    nc.vector.tensor_copy(den_sb[:], ps_den[:])
                    av_sb = sb.tile([128, ST], F32, tag="av_sb", bufs=2)
                    nc.vector.tensor_copy(av_sb[:], ps_av[:])
                    recip = sb.tile([128, ST], F32, tag="recip", bufs=2)
                    nc.vector.reciprocal(recip[:], den_sb[:])
                    nc.vector.tensor_tensor(out_t[g][si][:], av_sb[:],
                                            recip[:], Alu.mult)

            # ---------- o_proj: y_partial = wo_colshard @ out ----------
            # chunked: after each group of 8 row-blocks, ReduceScatter that
            # chunk (overlaps the collective with the next group's compute)
            NCHUNK = G  # 4 chunks of 8 row-blocks
            MO_PER = MO // NCHUNK
            cc_in = dram.tile([MO, 128, S], F32)
            cc_out = dram.tile([NCHUNK, 128, S], F32)
            for chunk in range(NCHUNK):
                for mo in range(chunk * MO_PER, (chunk + 1) * MO_PER):
                    wot = sb.tile([128, G, 128], F32R, tag="wo", bufs=4)
                    # scalar-engine DMA queue: keeps these reads from queuing
                    # behind the y-tile writes on the sync queue
                    nc.scalar.dma_start(wot[:], wo[mo])
                    for si in range(NST):
                        ps_y = ps.tile([128, ST], F32, tag="mm", bufs=4)
                        for g in range(G):
                            nc.tensor.matmul(ps_y[:], wot[:, g, :],
                                             out_t[g][si][:],
                                             start=(g == 0), stop=(g == G - 1))
                        y_sb = sb.tile([128, ST], F32, tag="y_sb", bufs=4)
                        dst = cc_in[mo][:, si * ST:(si + 1) * ST]
                        # split evictions AND their writeback DMA queues so
                        # y-writes never back up a single queue
                        if (mo + si) % 2 == 0:
                            nc.scalar.activation(y_sb[:], ps_y[:], ActF.Copy)
                            nc.gpsimd.dma_start(dst, y_sb[:])
                        else:
                            nc.vector.tensor_copy(y_sb[:], ps_y[:])
                            nc.sync.dma_start(dst, y_sb[:])
                if with_collective:
                    # core c receives row-block mo = chunk*8 + c
                    nc.gpsimd.collective_compute(
                        "ReduceScatter",
                        Alu.add,
                        replica_groups=[list(range(NCORES))],
                        ins=[cc_in[chunk * MO_PER:(chunk + 1) * MO_PER].opt()],
                        outs=[cc_out[chunk:chunk + 1].opt()],
                    )
                    nc.sync.dma_start(yout[chunk:chunk + 1], cc_out[chunk:chunk + 1])
            if not with_collective:
                # profiling-only variant: local copy instead of the
                # collective (output is the unreduced local shard)
                nc.sync.dma_start(yout[:, :, :], cc_in[MO - G:MO])

    nc.compile()
    return nc


def _rot_matrix():
    # q_rot = R @ q with rotate_half along D: R @ v = concat(-v[64:], v[:64])
    R = np.zeros((128, 128), np.float32)
    for i in range(64):
        R[i, 64 + i] = -1.0
        R[64 + i, i] = 1.0
    return R


def _prep_in_maps(inputs):
    x = np.ascontiguousarray(np.asarray(inputs["hidden_states"],
                                        np.float32)[0, :, 0, :])
    wq = np.asarray(inputs["wq"], np.float32)
    wk = np.asarray(inputs["wk"], np.float32)
    wv = np.asarray(inputs["wv"], np.float32)
    wo = np.asarray(inputs["wo"], np.float32)
    bq = np.asarray(inputs["bq"], np.float32)
    bk = np.asarray(inputs["bk"], np.float32)
    bv = np.asarray(inputs["bv"], np.float32)
    cos_t = np.ascontiguousarray(np.asarray(inputs["cos_t"],
                                            np.float32)[0, 0])  # (128, S)
    sin_t = np.ascontiguousarray(np.asarray(inputs["sin_t"], np.float32)[0, 0])
    rotT = np.ascontiguousarray(_rot_matrix().T)

    x_r = np.ascontiguousarray(x.reshape(KO, 128, S))
    in_maps = []
    for c in range(NCORES):
        qs = slice(c * G * 128, (c + 1) * G * 128)
        ks = slice(c * 128, (c + 1) * 128)
        wq_t = np.ascontiguousarray(wq[qs].T.reshape(KO, 128, G * 128))
        wk_t = np.ascontiguousarray(
            wk[ks].T.reshape(KO, 128, 128).transpose(1, 0, 2))
        wv_t = np.ascontiguousarray(
            wv[ks].T.reshape(KO, 128, 128).transpose(1, 0, 2))
        # wo column shard -> (mo, d, g, m): woT[g*128+d, mo*128+m]
        wo_t = np.ascontiguousarray(
            wo[:, qs].T.reshape(G, 128, MO, 128).transpose(2, 1, 0, 3))
        in_maps.append({
            "x": x_r,
            "wq": wq_t,
            "wk": wk_t,
            "wv": wv_t,
            "wo": wo_t,
            "bq": np.ascontiguousarray(bq[qs].reshape(G, 128).T),
            "bk": np.ascontiguousarray(bk[ks][:, None]),
            "bv": np.ascontiguousarray(bv[ks][:, None]),
            "cos": cos_t,
            "sin": sin_t,
            "rot": rotT,
        })
    return in_maps


_NC = None


def _get_nc():
    global _NC
    if _NC is None:
        _NC = build_nc()
    return _NC


def assemble_output(results):
    """Chunked ReduceScatter: core c's chunk i is y row-block mo = 8*i + c."""
    y = np.empty((HID, S), np.float32)
    for c in range(NCORES):
        yc = results[c]["y"]
        for i in range(yc.shape[0]):
            mo = NCORES * i + c
            y[mo * 128:(mo + 1) * 128] = yc[i]
    return y[None, :, None, :]


def kernel(**inputs):
    nc = _get_nc()
    in_maps = _prep_in_maps(inputs)
    res = run_bass_kernel_spmd(nc, in_maps, core_ids=list(range(NCORES)))
    return assemble_output(res.results)



# revision 12
# speedup vs baseline: 1.0780x; 1.0780x over previous
"""Trainium2 Bass kernel for AttentionPatcher (GQA attention block, S=2048).

Sharding: 8-way tensor parallel over KV head groups. Core c owns KV head c
and query heads 4c..4c+3: it computes its Q/K/V projections, RoPE, causal
attention, and a full partial o_proj (wo column shard); a per-s-tile
ReduceScatter(add) over the 8 cores then leaves core c with y row-blocks
4c..4c+3 of each s-slice, which the host reassembles.

All matmul operands are fp16 (1 cycle/row on the PE, same as fp32r, but
half the DMA/SBUF footprint and ~100x better precision than needed), with
fp32 PSUM accumulation. Weights are fully SBUF-resident; x is resident per
s-tile. The PE stream is software-pipelined per s-tile:
  QKV(si) -> attention(si-1) -> rope/vT(si) -> o_proj(si-1)
so attention and o_proj fill what would otherwise be PE idle gaps, and
only attention(3)+o_proj(3) remain as the tail.
"""
import os
import sys

import numpy as np

if os.path.isdir("/opt/trn_rl_repo") and "/opt/trn_rl_repo" not in sys.path:
    sys.path.insert(0, "/opt/trn_rl_repo")

import concourse.bacc as bacc
import concourse.mybir as mybir
import concourse.tile as tile
from concourse.bass_utils import run_bass_kernel_spmd
from concourse.masks import make_identity

F32 = mybir.dt.float32
F16 = mybir.dt.float16
ActF = mybir.ActivationFunctionType
Alu = mybir.AluOpType

H, KV, D, S = 32, 8, 128, 2048
HID = H * D
NCORES = 8
G = H // KV          # query heads per core
ST = 512             # s-tile size
NST = S // ST        # 4 s-tiles
KO = HID // 128      # 32 contraction subtiles
MO = HID // 128      # 32 output row tiles
INV_SQRT_D = 1.0 / float(np.sqrt(D))
EXP_BIAS = -2.0      # constant logit shift: keeps p=exp(s-2) in fp16 range


def build_nc(with_collective=True):
    nc = bacc.Bacc("TRN2", target_bir_lowering=False, debug=False)

    x = nc.dram_tensor("x", [128, KO, S], F16, kind="ExternalInput")
    wq = nc.dram_tensor("wq", [G, 128, KO, 128], F16, kind="ExternalInput")
    wk = nc.dram_tensor("wk", [128, KO, 128], F16, kind="ExternalInput")
    wv = nc.dram_tensor("wv", [128, KO, 128], F16, kind="ExternalInput")
    wo = nc.dram_tensor("wo", [G, 128, MO, 128], F16, kind="ExternalInput")
    bqs = nc.dram_tensor("bqs", [128, G], F32, kind="ExternalInput")
    bk = nc.dram_tensor("bk", [128, 1], F32, kind="ExternalInput")
    bv = nc.dram_tensor("bv", [128, 1], F32, kind="ExternalInput")
    cos = nc.dram_tensor("cos", [128, S], F16, kind="ExternalInput")
    sin = nc.dram_tensor("sin", [128, S], F16, kind="ExternalInput")
    rot = nc.dram_tensor("rot", [128, 128], F16, kind="ExternalInput")
    yout = nc.dram_tensor("y", [NST, MO // NCORES, 128, ST], F16,
                          kind="ExternalOutput")

    with tile.TileContext(nc) as tc:
        with (
            tc.tile_pool(name="const", bufs=1) as const,
            tc.tile_pool(name="sb", bufs=3) as sb,
            tc.tile_pool(name="ps", bufs=1, space="PSUM") as ps,
            tc.tile_pool(name="dram", bufs=1, space="DRAM") as dram,
        ):
            # ---------------- resident tensors ----------------
            wq_sb = [const.tile([128, KO, 128], F16, name=f"wq{g}")
                     for g in range(G)]
            wk_sb = const.tile([128, KO, 128], F16)
            wv_sb = const.tile([128, KO, 128], F16)
            wo_sb = [const.tile([128, MO, 128], F16, name=f"wo{g}")
                     for g in range(G)]
            cos_sb = const.tile([128, S], F16)
            sin_sb = const.tile([128, S], F16)
            rot_sb = const.tile([128, 128], F16)
            bq_sb = const.tile([128, G], F32)
            bk_sb = const.tile([128, 1], F32)
            bv_sb = const.tile([128, 1], F32)
            k_rot = const.tile([128, S], F16)             # K, (d, l) layout
            v_t = const.tile([128, S // 128, 128], F16)   # V^T (l%128, l//128, d)
            out_t = [[const.tile([128, ST], F16, name=f"out_{g}_{si}")
                      for si in range(NST)] for g in range(G)]

            # small constants first (tiny, scalar queue)
            nc.scalar.dma_start(bq_sb[:], bqs[:, :])
            nc.scalar.dma_start(bk_sb[:], bk[:, :])
            nc.scalar.dma_start(bv_sb[:], bv[:, :])
            nc.scalar.dma_start(rot_sb[:], rot[:, :])

            ones16 = const.tile([128, 128], F16)
            nc.any.memset(ones16[:], 1.0)
            expb = const.tile([128, 1], F32)
            nc.any.memset(expb[:], EXP_BIAS)
            ident16 = const.tile([128, 128], F16)
            make_identity(nc, ident16)

            # si=0 weights + x arrive ko-chunk-interleaved so the ko-outer
            # first QKV phase streams without waiting for full loads
            xq = [nc.sync, nc.scalar, nc.gpsimd]
            x_pool_tiles = {}
            x_pool_tiles[0] = sb.tile([128, KO, ST], F16, tag="x", bufs=2, name="x0")
            sl0 = slice(0, ST)
            for r in range(8):
                ks = slice(4 * r, 4 * r + 4)
                nc.sync.dma_start(wk_sb[:, ks, :], wk[:, ks, :])
                nc.scalar.dma_start(wv_sb[:, ks, :], wv[:, ks, :])
                for g in range(G):
                    nc.gpsimd.dma_start(wq_sb[g][:, ks, :], wq[g][:, ks, :])
                ka = slice(4 * r, 4 * r + 2)
                kb = slice(4 * r + 2, 4 * r + 4)
                nc.sync.dma_start(x_pool_tiles[0][:, ka, :], x[:, ka, sl0])
                nc.scalar.dma_start(x_pool_tiles[0][:, kb, :], x[:, kb, sl0])
            # bulk prefetch (needed from rope(0) / o_proj(0) onwards)
            nc.sync.dma_start(cos_sb[:], cos[:, :])
            nc.scalar.dma_start(sin_sb[:], sin[:, :])
            for g in range(G):
                xq[g % 3].dma_start(wo_sb[g][:], wo[g][:, :, :])

            cc_in = dram.tile([NST, MO, 128, ST], F16)
            cc_out = dram.tile([NST, MO // NCORES, 128, ST], F16)

            # ---------------- phase helpers ----------------
            k_raws, v_sbs, q_raws, q_rots = {}, {}, {}, {}

            def prefetch_x(si):
                t = sb.tile([128, KO, ST], F16, tag="x", bufs=2,
                             name=f"x{si}")
                x_pool_tiles[si] = t
                sl = slice(si * ST, (si + 1) * ST)
                for r in range(8):
                    ka = slice(4 * r, 4 * r + 2)
                    kb = slice(4 * r + 2, 4 * r + 4)
                    xq[r % 3].dma_start(t[:, ka, :], x[:, ka, sl])
                    xq[(r + 1) % 3].dma_start(t[:, kb, :], x[:, kb, sl])

            def evict_q(ps_q, g, si):
                q_raw = sb.tile([128, ST], F16, tag="qraw", bufs=5,
                                name=f"q_raw{g}")
                # (q + bq) * inv_sqrt_d, with bq pre-scaled on host
                nc.scalar.activation(q_raw[:], ps_q[:], ActF.Identity,
                                     bias=bq_sb[:, g:g + 1], scale=INV_SQRT_D)
                q_raws[(si, g)] = q_raw

            def evict_k(ps_k, si):
                k_raw = sb.tile([128, ST], F16, tag="kraw", bufs=2)
                nc.scalar.activation(k_raw[:], ps_k[:], ActF.Identity,
                                     bias=bk_sb[:, 0:1])
                k_raws[si] = k_raw

            def evict_v(ps_v, si):
                v_sb = sb.tile([128, ST], F16, tag="vsb", bufs=2)
                nc.vector.tensor_scalar(v_sb[:], ps_v[:], bv_sb[:, 0:1], None,
                                        Alu.add)
                v_sbs[si] = v_sb

            def qkv0():
                """si=0: ko-outer (6 live accumulators) so compute streams
                at the DMA arrival rate of the interleaved weight chunks."""
                xs = x_pool_tiles[0]
                ps_k = ps.tile([128, ST], F32, tag="acc", bufs=2)
                ps_v = ps.tile([128, ST], F32, tag="acc", bufs=2)
                ps_q = [ps.tile([128, ST], F32, tag="s", bufs=3,
                                name=f"ps_q{i}") for i in range(3)]
                ps_q.append(ps.tile([128, ST], F32, tag="den", bufs=1,
                                    name="ps_q3"))
                for ko in range(KO):
                    st = (ko == 0)
                    sp = (ko == KO - 1)
                    nc.tensor.matmul(ps_k[:], wk_sb[:, ko, :], xs[:, ko, :],
                                     start=st, stop=sp)
                    nc.tensor.matmul(ps_v[:], wv_sb[:, ko, :], xs[:, ko, :],
                                     start=st, stop=sp)
                    for g in range(G):
                        nc.tensor.matmul(ps_q[g][:], wq_sb[g][:, ko, :],
                                         xs[:, ko, :], start=st, stop=sp)
                evict_k(ps_k, 0)
                evict_v(ps_v, 0)
                for g in range(G):
                    evict_q(ps_q[g], g, 0)

            def qkv(si):
                """si>0: head-outer (1-2 live accumulators; weights and x all
                SBUF-resident so the PE streams back-to-back)."""
                xs = x_pool_tiles[si]

                def block(w_sb, evict):
                    acc = ps.tile([128, ST], F32, tag="acc", bufs=2)
                    for ko in range(KO):
                        nc.tensor.matmul(acc[:], w_sb[:, ko, :], xs[:, ko, :],
                                         start=(ko == 0), stop=(ko == KO - 1))
                    evict(acc)

                block(wk_sb, lambda a: evict_k(a, si))
                block(wv_sb, lambda a: evict_v(a, si))
                for g in range(G):
                    block(wq_sb[g], lambda a, g=g: evict_q(a, g, si))

            def rope_phase(si):
                sl = slice(si * ST, (si + 1) * ST)
                # V^T into v_t: 4 PE transposes packed into one psum bank
                # (start only on the first so later ones don't re-zero it)
                v_sb = v_sbs.pop(si)
                nj = ST // 128
                ps_t = ps.tile([128, nj, 128], F16, tag="vt", bufs=1)
                for j in range(nj):
                    nc.tensor.matmul(ps_t[:, j, :],
                                     v_sb[:, j * 128:(j + 1) * 128],
                                     ident16[:], is_transpose=True,
                                     start=(j == 0), stop=(j == nj - 1))
                nc.vector.tensor_copy(v_t[:, si * nj:(si + 1) * nj, :],
                                      ps_t[:])

                def rope(raw, dst_ap):
                    """dst = raw*cos + (rot@raw)*sin over s-slice sl."""
                    ps_r = ps.tile([128, ST], F32, tag="acc", bufs=2)
                    nc.tensor.matmul(ps_r[:], rot_sb[:], raw[:],
                                     start=True, stop=True)
                    t1 = sb.tile([128, ST], F16, tag="rope_t1", bufs=2)
                    t2 = sb.tile([128, ST], F16, tag="rope_t2", bufs=2)
                    nc.gpsimd.tensor_tensor(t1[:], raw[:], cos_sb[:, sl],
                                            Alu.mult)
                    nc.vector.tensor_tensor(t2[:], ps_r[:], sin_sb[:, sl],
                                            Alu.mult)
                    nc.vector.tensor_tensor(dst_ap, t1[:], t2[:], Alu.add)

                rope(k_raws.pop(si), k_rot[:, sl])
                for g in range(G):
                    q_rot = sb.tile([128, ST], F16, tag="qrot", bufs=8,
                                    name=f"q_rot{g}")
                    rope(q_raws.pop((si, g)), q_rot[:])
                    q_rots[(si, g)] = q_rot

            def attn(si):
                nli = (si + 1) * (ST // 128)
                for g in range(G):
                    q_rot = q_rots.pop((si, g))
                    ps_av = ps.tile([128, ST], F32, tag="av", bufs=1)
                    ps_den = ps.tile([128, ST], F32, tag="den", bufs=1)
                    # emit with 2-deep score lookahead so the PE never waits
                    # on the exp->select chain of the block it consumes
                    pend = []

                    def score(li):
                        j = li - si * (ST // 128)
                        off = 128 * j if j > 0 else 0
                        ps_s = ps.tile([128, ST], F32, tag="s", bufs=3)
                        nc.tensor.matmul(ps_s[:, off:],
                                         k_rot[:, li * 128:(li + 1) * 128],
                                         q_rot[:, off:], start=True, stop=True)
                        p = sb.tile([128, ST], F16, tag="p", bufs=3)
                        nc.scalar.activation(p[:, off:], ps_s[:, off:],
                                             ActF.Exp, bias=expb[:, 0:1])
                        if j >= 0:
                            # causal triangle only lives in the first 128
                            # cols after the diagonal offset
                            nc.gpsimd.affine_select(
                                out=p[:, off:off + 128],
                                in_=p[:, off:off + 128],
                                compare_op=Alu.is_ge, fill=0.0,
                                base=0, channel_multiplier=-1,
                                pattern=[[1, 128]],
                            )
                        return li, off, p

                    def accum(li, off, p):
                        nc.tensor.matmul(ps_av[:, off:], v_t[:, li, :],
                                         p[:, off:], start=(li == 0),
                                         stop=(li == nli - 1))
                        nc.tensor.matmul(ps_den[:, off:], ones16[:],
                                         p[:, off:], start=(li == 0),
                                         stop=(li == nli - 1))

                    for li in range(nli):
                        pend.append(score(li))
                        if len(pend) > 2:
                            accum(*pend.pop(0))
                    for it in pend:
                        accum(*it)

                    recip = sb.tile([128, ST], F32, tag="recip", bufs=1)
                    nc.vector.reciprocal_approx_fast(recip[:], ps_den[:])
                    nc.vector.tensor_tensor(out_t[g][si][:], ps_av[:],
                                            recip[:], Alu.mult)

            def oproj(si):
                for mo in range(MO):
                    ps_y = ps.tile([128, ST], F32, tag="acc", bufs=2)
                    for g in range(G):
                        nc.tensor.matmul(ps_y[:], wo_sb[g][:, mo, :],
                                         out_t[g][si][:],
                                         start=(g == 0), stop=(g == G - 1))
                    y_sb = sb.tile([128, ST], F16, tag="y", bufs=3)
                    dst = cc_in[si, mo]
                    # split evictions and writeback queues so y writes never
                    # back up a single engine/queue
                    if mo % 2 == 0:
                        nc.scalar.activation(y_sb[:], ps_y[:], ActF.Copy)
                        nc.gpsimd.dma_start(dst, y_sb[:])
                    else:
                        nc.vector.tensor_copy(y_sb[:], ps_y[:])
                        nc.sync.dma_start(dst, y_sb[:])
                if with_collective:
                    # core c receives y row-blocks 4c..4c+3 of this s-slice
                    nc.gpsimd.collective_compute(
                        "ReduceScatter",
                        Alu.add,
                        replica_groups=[list(range(NCORES))],
                        ins=[cc_in[si].opt()],
                        outs=[cc_out[si].opt()],
                    )
                    nc.sync.dma_start(yout[si], cc_out[si])
                else:
                    # profiling-only variant: local copy instead of the
                    # collective (output is an unreduced local shard)
                    nc.sync.dma_start(yout[si], cc_in[si, 0:MO // NCORES])

            # ---------------- pipelined schedule ----------------
            qkv0()
            rope_phase(0)
            prefetch_x(1)
            for si in range(1, NST):
                qkv(si)
                if si + 1 < NST:
                    prefetch_x(si + 1)
                attn(si - 1)
                rope_phase(si)
                oproj(si - 1)
            attn(NST - 1)
            oproj(NST - 1)

    nc.compile()
    return nc


def _rot_matrix():
    # q_rot = R @ q with rotate_half along D: R @ v = concat(-v[64:], v[:64])
    R = np.zeros((128, 128), np.float32)
    for i in range(64):
        R[i, 64 + i] = -1.0
        R[64 + i, i] = 1.0
    return R


def _prep_in_maps(inputs):
    f16 = np.float16
    x = np.asarray(inputs["hidden_states"], np.float32)[0, :, 0, :]
    wq = np.asarray(inputs["wq"], np.float32)
    wk = np.asarray(inputs["wk"], np.float32)
    wv = np.asarray(inputs["wv"], np.float32)
    wo = np.asarray(inputs["wo"], np.float32)
    bq = np.asarray(inputs["bq"], np.float32)
    bk = np.asarray(inputs["bk"], np.float32)
    bv = np.asarray(inputs["bv"], np.float32)
    cos_t = np.asarray(inputs["cos_t"], np.float32)[0, 0]  # (128, S)
    sin_t = np.asarray(inputs["sin_t"], np.float32)[0, 0]

    x_r = np.ascontiguousarray(
        x.reshape(KO, 128, S).transpose(1, 0, 2).astype(f16))
    cos16 = np.ascontiguousarray(cos_t.astype(f16))
    sin16 = np.ascontiguousarray(sin_t.astype(f16))
    rotT = np.ascontiguousarray(_rot_matrix().T.astype(f16))

    in_maps = []
    for c in range(NCORES):
        qs = slice(c * G * 128, (c + 1) * G * 128)
        ks = slice(c * 128, (c + 1) * 128)
        # lhsT layouts: [g][kk, ko, m] for wq, [kk, ko, m] for wk/wv,
        # [g][d, mo, m] for wo
        wq_t = np.ascontiguousarray(
            wq[qs].T.reshape(KO, 128, G, 128).transpose(2, 1, 0, 3)
            .astype(f16))
        wk_t = np.ascontiguousarray(
            wk[ks].T.reshape(KO, 128, 128).transpose(1, 0, 2).astype(f16))
        wv_t = np.ascontiguousarray(
            wv[ks].T.reshape(KO, 128, 128).transpose(1, 0, 2).astype(f16))
        wo_t = np.ascontiguousarray(
            wo[:, qs].T.reshape(G, 128, MO, 128).astype(f16))
        in_maps.append({
            "x": x_r,
            "wq": wq_t,
            "wk": wk_t,
            "wv": wv_t,
            "wo": wo_t,
            # bq pre-scaled by 1/sqrt(D): eviction computes q*s + bq*s
            "bqs": np.ascontiguousarray(
                (bq[qs] * INV_SQRT_D).reshape(G, 128).T.astype(np.float32)),
            "bk": np.ascontiguousarray(bk[ks][:, None]),
            "bv": np.ascontiguousarray(bv[ks][:, None]),
            "cos": cos16,
            "sin": sin16,
            "rot": rotT,
        })
    return in_maps


_NC = None


def _get_nc():
    global _NC
    if _NC is None:
        _NC = build_nc()
    return _NC


def assemble_output(results):
    """Per-s-tile ReduceScatter: core c's row i of chunk si is y row-block
    4c+i over s-slice si."""
    y = np.empty((HID, S), np.float32)
    npc = MO // NCORES
    for c in range(NCORES):
        yc = results[c]["y"]  # [NST, 4, 128, ST] f16
        for si in range(NST):
            for i in range(npc):
                mo = npc * c + i
                y[mo * 128:(mo + 1) * 128, si * ST:(si + 1) * ST] = yc[si, i]
    return y[None, :, None, :]


def kernel(**inputs):
    nc = _get_nc()
    in_maps = _prep_in_maps(inputs)
    res = run_bass_kernel_spmd(nc, in_maps, core_ids=list(range(NCORES)))
    return assemble_output(res.results)
